# revision 12
# baseline (speedup 1.0000x reference)
"""Trainium2 Bass kernel for nn_ContrastiveLoss (exp-cosine ranking loss).

Math: sort rows of output1 by descending ranking (stable). With
e_b[i] = exp(cos_sim(x_sorted[i], o_b)) for b in {2,3} and suffix sums
suf_b(i) = sum_{j>=i} e_b[j], the reference loss equals

    loss = N*(log T2 + log T3) - sum_i log suf2(i) - sum_i log suf3(i)

where T_b = suf_b(0) is the global total.  Sharding: host sorts by
ranking (the sort defines the shard boundaries, i.e. shards are
rank-contiguous) and feeds rows in ASCENDING rank order so forward
cumsums on-device are exactly the suffix sums of the reference order.
Each core handles 8192 rows: computes per-row exp-cosines, local
cumsums, all-gathers the 8 per-shard totals to form global bases, then
does the log-reduction.  A second tiny AllGather makes every core emit
the same final scalar.

Engine split in the streaming phase (per [128, 512] row-tile):
  DVE:    affine_mul_reduce -> d2 (fused dot), tensor_reduce of p3 -> d3
  GpSimd: p3 = x * o3 (elementwise)
  ACT:    Square + accumulate -> row sum-of-squares
The per-shard scan machinery runs before/during the AllGather wait; the
global base lands as the per-partition bias of the final Ln activation.
"""

import numpy as np

N, D = 65536, 512
NCORES = 8
SH = N // NCORES            # 8192 rows per core
TPC = SH // 128             # 64 row-tiles of 128 per core
BLK = 8                     # row-tiles per DMA (1MB per transfer)
NBLK = TPC // BLK           # 8 DMA blocks

_compiled_nc = None


def _body(tc, mybir, masks, xs, o2, o3, mlt, is0, loss_out):
    """Emit the per-core Tile kernel. All args are bass.APs of DRAM tensors."""
    nc = tc.nc
    f32 = mybir.dt.float32
    OP = mybir.AluOpType
    AF = mybir.ActivationFunctionType
    AX = mybir.AxisListType

    with (
        tc.tile_pool(name="const", bufs=1) as constp,
        tc.tile_pool(name="xin", bufs=4) as xinp,
        tc.tile_pool(name="scr", bufs=2) as scrp,
        tc.tile_pool(name="p3pool", bufs=3) as p3p,
        tc.tile_pool(name="stats", bufs=1) as statsp,
        tc.tile_pool(name="small", bufs=1) as smallp,
        tc.tile_pool(name="psum", bufs=1, space="PSUM") as psump,
        tc.tile_pool(name="dram", bufs=1, space="DRAM") as dramp,
    ):
        # ---- constants ----
        o2b = constp.tile([128, D], f32)
        nc.sync.dma_start(o2b[:], o2.broadcast_to((128, D)))
        o3b = constp.tile([128, D], f32)
        nc.sync.dma_start(o3b[:], o3.broadcast_to((128, D)))
        mltt = constp.tile([8, 128], f32)
        nc.sync.dma_start(mltt[:], mlt)
        is0t = constp.tile([1, 1], f32)
        nc.sync.dma_start(is0t[:], is0)
        ident = constp.tile([128, 128], f32)
        masks.make_identity(nc, ident[:])
        ones128 = constp.tile([128, 1], f32)
        nc.vector.memset(ones128[:], 1.0)

        # warm the ACT table store for every function used later so the
        # tail pays no cold table loads (the store caches multiple tables)
        warm = constp.tile([1, 1], f32)
        nc.vector.memset(warm[:], 1.0)
        wsc = constp.tile([1, 1], f32)
        for fn in (AF.Square, AF.Sqrt, AF.Exp, AF.Ln, AF.Copy):
            nc.scalar.activation(wsc[:], warm[:], fn)

        # 1/||o2||, 1/||o3|| replicated on every partition
        sco = scrp.tile([128, D], f32, tag="actscr")
        so2 = smallp.tile([128, 1], f32)
        nc.scalar.activation(sco[:], o2b[:], AF.Square, accum_out=so2[:])
        n2b = smallp.tile([128, 1], f32)
        nc.scalar.activation(n2b[:], so2[:], AF.Sqrt)
        invn2b = smallp.tile([128, 1], f32)
        nc.vector.reciprocal(invn2b[:], n2b[:])
        sco2 = scrp.tile([128, D], f32, tag="actscr")
        so3 = smallp.tile([128, 1], f32)
        nc.scalar.activation(sco2[:], o3b[:], AF.Square, accum_out=so3[:])
        n3b = smallp.tile([128, 1], f32)
        nc.scalar.activation(n3b[:], so3[:], AF.Sqrt)
        invn3b = smallp.tile([128, 1], f32)
        nc.vector.reciprocal(invn3b[:], n3b[:])

        # ---- phase 1: per-row dots and square-sums over the 16MiB shard ----
        d2all = statsp.tile([128, TPC], f32)
        d3all = statsp.tile([128, TPC], f32)
        ssall = statsp.tile([128, TPC], f32)

        xv = xs.rearrange("(b k p) d -> b p k d", p=128, k=BLK)  # [NBLK,128,BLK,D]
        for b in range(NBLK):
            xt = xinp.tile([128, BLK, D], f32)
            nc.sync.dma_start(xt[:], xv[b])
            for k in range(BLK):
                t = b * BLK + k
                s1 = scrp.tile([128, D], f32, tag="ttrscr")
                nc.vector.affine_mul_reduce(
                    out=s1[:], accum_out=d2all[:, t : t + 1],
                    in0=xt[:, k, :], in1=o2b[:], scale=1.0, bias=0.0)
                p3 = p3p.tile([128, D], f32)
                nc.gpsimd.tensor_tensor(
                    out=p3[:], in0=xt[:, k, :], in1=o3b[:], op=OP.mult)
                nc.vector.tensor_reduce(
                    out=d3all[:, t : t + 1], in_=p3[:], axis=AX.X, op=OP.add)
                s3 = scrp.tile([128, D], f32, tag="actscr")
                nc.scalar.activation(
                    s3[:], xt[:, k, :], AF.Square, accum_out=ssall[:, t : t + 1])

        # ---- phase 2: exp-cosines ----
        nrm = statsp.tile([128, TPC], f32)
        nc.scalar.activation(nrm[:], ssall[:], AF.Sqrt)
        rs = statsp.tile([128, TPC], f32)
        nc.vector.reciprocal(rs[:], nrm[:])
        t2 = statsp.tile([128, TPC], f32)
        nc.vector.tensor_tensor(out=t2[:], in0=d2all[:], in1=rs[:], op=OP.mult)
        t3 = statsp.tile([128, TPC], f32)
        nc.vector.tensor_tensor(out=t3[:], in0=d3all[:], in1=rs[:], op=OP.mult)
        # eall[:, 0:64] = e2 per (row p, tile t); eall[:, 64:128] = e3
        eall = statsp.tile([128, 2 * TPC], f32)
        nc.scalar.activation(eall[:, 0:TPC], t2[:], AF.Exp, scale=invn2b[:])
        nc.scalar.activation(eall[:, TPC:], t3[:], AF.Exp, scale=invn3b[:])

        # ---- phase 3a: local totals -> post the AllGather as early as possible
        # per-(branch,tile) totals, row layout: totr[0, q] = sum_p eall[p, q]
        totr_ps = psump.tile([1, 128], f32)
        nc.tensor.matmul(totr_ps[:], ones128[:], eall[:], start=True, stop=True)
        totr = smallp.tile([1, 128], f32)
        nc.vector.tensor_copy(totr[:], totr_ps[:])
        tl = smallp.tile([1, 2], f32)
        nc.vector.tensor_reduce(out=tl[:, 0:1], in_=totr[:, 0:TPC], axis=AX.X, op=OP.add)
        nc.vector.tensor_reduce(out=tl[:, 1:2], in_=totr[:, TPC:], axis=AX.X, op=OP.add)
        cc_in = dramp.tile([1, 2], f32)
        cc_out = dramp.tile([8, 2], f32, addr_space="Shared")
        nc.sync.dma_start(cc_in[:], tl[:])
        nc.gpsimd.collective_compute(
            "AllGather", OP.bypass, replica_groups=[list(range(NCORES))],
            ins=[cc_in.opt()], outs=[cc_out.opt()])

        # ---- phase 3b: shard-local scans (overlap the AllGather skew wait)
        # transpose -> eT[q, p] with q = branch*64 + t
        eT_ps = psump.tile([128, 128], f32)
        nc.tensor.transpose(eT_ps[:], eall[:], ident[:])
        eT = statsp.tile([128, 128], f32)
        nc.scalar.copy(eT[:], eT_ps[:])
        # shifted (exclusive) tile totals, local only
        sh = smallp.tile([1, 128], f32)
        nc.vector.memset(sh[:, 0:1], 0.0)
        nc.vector.memset(sh[:, TPC : TPC + 1], 0.0)
        nc.vector.tensor_copy(sh[:, 1:TPC], totr[:, 0 : TPC - 1])
        nc.vector.tensor_copy(sh[:, TPC + 1 :], totr[:, TPC : 2 * TPC - 1])
        baser = smallp.tile([1, 128], f32)
        nc.vector.tensor_tensor_scan(
            out=baser[:, 0:TPC], data0=sh[:, 0:TPC], data1=sh[:, 0:TPC],
            initial=0.0, op0=OP.add, op1=OP.bypass)
        nc.vector.tensor_tensor_scan(
            out=baser[:, TPC:], data0=sh[:, TPC:], data1=sh[:, TPC:],
            initial=0.0, op0=OP.add, op1=OP.bypass)
        # move per-tile bases onto partitions: basec[q, 0] = baser[0, q]
        basec = smallp.tile([128, 1], f32)
        nc.sync.dma_start(basec[:], baser[:])
        # inclusive scan within each tile (along p) seeded by the local base:
        # sufl[q, p] = local suffix sums (missing only the global core base)
        sufl = statsp.tile([128, 128], f32)
        nc.vector.tensor_tensor_scan(
            out=sufl[:], data0=eT[:], data1=eT[:], initial=basec[:],
            op0=OP.add, op1=OP.bypass)

        # ---- phase 3c: consume the AllGather ----
        ag = smallp.tile([8, 2], f32)
        nc.sync.dma_start(ag[:], cc_out[:])
        # per-partition global bases: gb_ps[q, b] = sum_{c < my_core} tot_b[c]
        gb_ps = psump.tile([128, 2], f32)
        nc.tensor.matmul(gb_ps[:], mltt[:], ag[:], start=True, stop=True)
        tg_ps = psump.tile([1, 2], f32)
        nc.tensor.matmul(tg_ps[:], ones128[0:8, :], ag[:], start=True, stop=True)

        gb = smallp.tile([128, 2], f32)
        nc.vector.tensor_copy(gb[:], gb_ps[:])

        # ---- phase 4: log-reduction (global base folded into Ln bias) ----
        lnscr = statsp.tile([128, 128], f32)
        lnacc = smallp.tile([128, 1], f32)
        nc.scalar.activation(lnscr[0:TPC, :], sufl[0:TPC, :], AF.Ln,
                             bias=gb[0:TPC, 0:1], accum_out=lnacc[0:TPC, :])
        nc.scalar.activation(lnscr[TPC:, :], sufl[TPC:, :], AF.Ln,
                             bias=gb[TPC:, 1:2], accum_out=lnacc[TPC:, :])
        part_ps = psump.tile([1, 1], f32)
        nc.tensor.matmul(part_ps[:], ones128[:], lnacc[:], start=True, stop=True)

        # local partial = is0 * N * (log T2 + log T3) - sum(log suf)
        lt = smallp.tile([1, 2], f32)
        nc.scalar.activation(lt[:], tg_ps[:], AF.Ln)
        lts = smallp.tile([1, 1], f32)
        nc.vector.tensor_reduce(out=lts[:], in_=lt[:], axis=AX.X, op=OP.add)
        f1 = smallp.tile([1, 1], f32)
        nc.scalar.mul(f1[:], lts[:], float(N))
        f2 = smallp.tile([1, 1], f32)
        nc.vector.tensor_tensor(out=f2[:], in0=f1[:], in1=is0t[:], op=OP.mult)
        f3 = smallp.tile([1, 1], f32)
        nc.vector.tensor_tensor(out=f3[:], in0=f2[:], in1=part_ps[:], op=OP.subtract)

        # AllGather partials and reduce so every core emits the full loss
        cc2_in = dramp.tile([1, 1], f32)
        cc2_out = dramp.tile([8, 1], f32, addr_space="Shared")
        nc.sync.dma_start(cc2_in[:], f3[:])
        nc.gpsimd.collective_compute(
            "AllGather", OP.bypass, replica_groups=[list(range(NCORES))],
            ins=[cc2_in.opt()], outs=[cc2_out.opt()])
        agp = smallp.tile([8, 1], f32)
        nc.sync.dma_start(agp[:], cc2_out[:])
        fin_ps = psump.tile([1, 1], f32)
        nc.tensor.matmul(fin_ps[:], ones128[0:8, :], agp[:], start=True, stop=True)
        fin = smallp.tile([1, 1], f32)
        nc.vector.tensor_copy(fin[:], fin_ps[:])
        nc.sync.dma_start(loss_out[:], fin[:])


def build_nc():
    """Build + compile the SPMD Bass program (cached)."""
    global _compiled_nc
    if _compiled_nc is not None:
        return _compiled_nc
    import concourse.bacc as bacc
    import concourse.mybir as mybir
    from concourse import masks, tile

    f32 = mybir.dt.float32
    nc = bacc.Bacc("TRN2", target_bir_lowering=False, debug=False,
                   num_devices=NCORES)
    xs = nc.dram_tensor("xs", [SH, D], f32, kind="ExternalInput")
    o2 = nc.dram_tensor("o2", [1, D], f32, kind="ExternalInput")
    o3 = nc.dram_tensor("o3", [1, D], f32, kind="ExternalInput")
    mlt = nc.dram_tensor("mlt", [8, 128], f32, kind="ExternalInput")
    is0 = nc.dram_tensor("is0", [1, 1], f32, kind="ExternalInput")
    loss = nc.dram_tensor("loss", [1, 1], f32, kind="ExternalOutput")

    with tile.TileContext(nc) as tc:
        _body(tc, mybir, masks, xs.ap(), o2.ap(), o3.ap(), mlt.ap(), is0.ap(),
              loss.ap())
    nc.compile()
    _compiled_nc = nc
    return nc


def make_in_maps(output1, output2, output3, ranking):
    """Host-side shard: sort rows by descending ranking (stable, matching
    jnp.argsort(-ranking)), then feed in reversed (ascending) order so the
    device's forward cumsums are the reference's suffix sums."""
    ranking = np.asarray(ranking, dtype=np.float32)
    order = np.argsort(-ranking, kind="stable")
    rho = order[::-1]
    xs_full = np.ascontiguousarray(np.asarray(output1, dtype=np.float32)[rho])
    o2 = np.ascontiguousarray(np.asarray(output2, dtype=np.float32).reshape(1, D))
    o3 = np.ascontiguousarray(np.asarray(output3, dtype=np.float32).reshape(1, D))
    in_maps = []
    for c in range(NCORES):
        mlt = np.zeros((8, 128), np.float32)
        mlt[:c] = 1.0
        is0 = np.full((1, 1), 1.0 if c == 0 else 0.0, np.float32)
        in_maps.append({
            "xs": xs_full[c * SH : (c + 1) * SH],
            "o2": o2, "o3": o3, "mlt": mlt, "is0": is0,
        })
    return in_maps


def kernel(output1, output2, output3, ranking):
    from concourse.bass_utils import run_bass_kernel_spmd

    nc = build_nc()
    in_maps = make_in_maps(output1, output2, output3, ranking)
    res = run_bass_kernel_spmd(nc, in_maps, core_ids=list(range(NCORES)))
    out = res.results[0]["loss"]
    return np.asarray(out, dtype=np.float32).reshape(())


# revision 15
# speedup vs baseline: 1.2716x; 1.2716x over previous
"""Trainium2 Bass kernel for nn_ContrastiveLoss (exp-cosine ranking loss).

Math: sort rows of output1 by descending ranking (stable). With
e_b[i] = exp(cos_sim(x_sorted[i], o_b)) for b in {2,3} and suffix sums
suf_b(i) = sum_{j>=i} e_b[j], the reference loss equals

    loss = N*(log T2 + log T3) - sum_i log suf2(i) - sum_i log suf3(i)

where T_b = suf_b(0) is the global total.  Sharding: host sorts by
ranking (the sort defines the shard boundaries, i.e. shards are
rank-contiguous) and feeds rows in ASCENDING rank order so forward
cumsums on-device are exactly the suffix sums of the reference order.
Each core handles 8192 rows: computes per-row exp-cosines, local
cumsums, all-gathers the 8 per-shard totals to form global bases, then
does the log-reduction.  A second tiny AllGather makes every core emit
the same final scalar.

Engine split in the streaming phase (per [128, 512] row-tile):
  DVE:    affine_mul_reduce -> d2 (fused dot), tensor_reduce of p3 -> d3
  GpSimd: p3 = x * o3 (elementwise)
  ACT:    Square + accumulate -> row sum-of-squares
The per-shard scan machinery runs before/during the AllGather wait; the
global base lands as the per-partition bias of the final Ln activation.
"""

import numpy as np

N, D = 65536, 512
NCORES = 8
SH = N // NCORES            # 8192 rows per core
TPC = SH // 128             # 64 row-tiles of 128 per core
BLK = 8                     # row-tiles per DMA (1MB per transfer)
NBLK = TPC // BLK           # 8 DMA blocks

_compiled_nc = None


def _body(tc, mybir, masks, xs, o2, o3, mlt, is0, loss_out):
    """Emit the per-core Tile kernel. All args are bass.APs of DRAM tensors."""
    nc = tc.nc
    f32 = mybir.dt.float32
    OP = mybir.AluOpType
    AF = mybir.ActivationFunctionType
    AX = mybir.AxisListType

    with (
        tc.tile_pool(name="const", bufs=1) as constp,
        tc.tile_pool(name="xin", bufs=6) as xinp,
        tc.tile_pool(name="scr", bufs=2) as scrp,
        tc.tile_pool(name="stats", bufs=1) as statsp,
        tc.tile_pool(name="small", bufs=1) as smallp,
        tc.tile_pool(name="psum", bufs=1, space="PSUM") as psump,
        tc.tile_pool(name="dram", bufs=1, space="DRAM") as dramp,
    ):
        # ---- constants ----
        o2b = constp.tile([128, D], f32)
        nc.sync.dma_start(o2b[:], o2.broadcast_to((128, D)))
        o3b = constp.tile([128, D], f32)
        nc.sync.dma_start(o3b[:], o3.broadcast_to((128, D)))
        mltt = constp.tile([8, 128], f32)
        nc.sync.dma_start(mltt[:], mlt)
        is0t = constp.tile([1, 1], f32)
        nc.sync.dma_start(is0t[:], is0)
        ident = constp.tile([128, 128], f32)
        masks.make_identity(nc, ident[:])
        ones128 = constp.tile([128, 1], f32)
        nc.vector.memset(ones128[:], 1.0)

        # 1/||o2||, 1/||o3|| replicated on every partition
        sco = scrp.tile([128, D], f32, tag="actscr")
        so2 = smallp.tile([128, 1], f32)
        nc.scalar.activation(sco[:], o2b[:], AF.Square, accum_out=so2[:])
        n2b = smallp.tile([128, 1], f32)
        nc.scalar.activation(n2b[:], so2[:], AF.Sqrt)
        invn2b = smallp.tile([128, 1], f32)
        nc.vector.reciprocal(invn2b[:], n2b[:])
        sco2 = scrp.tile([128, D], f32, tag="actscr")
        so3 = smallp.tile([128, 1], f32)
        nc.scalar.activation(sco2[:], o3b[:], AF.Square, accum_out=so3[:])
        n3b = smallp.tile([128, 1], f32)
        nc.scalar.activation(n3b[:], so3[:], AF.Sqrt)
        invn3b = smallp.tile([128, 1], f32)
        nc.vector.reciprocal(invn3b[:], n3b[:])

        # ---- phase 1: per-row dots and square-sums over the 16MiB shard ----
        d2all = statsp.tile([128, TPC], f32)
        d3all = statsp.tile([128, TPC], f32)
        ssall = statsp.tile([128, TPC], f32)

        xv = xs.rearrange("(b k p) d -> b p k d", p=128, k=BLK)  # [NBLK,128,BLK,D]
        for b in range(NBLK):
            xt = xinp.tile([128, BLK, D], f32)
            nc.sync.dma_start(xt[:], xv[b])
            for k in range(BLK):
                t = b * BLK + k
                s1 = scrp.tile([128, D], f32, tag="ttrscr")
                nc.vector.affine_mul_reduce(
                    out=s1[:], accum_out=d2all[:, t : t + 1],
                    in0=xt[:, k, :], in1=o2b[:], scale=1.0, bias=0.0)
                s2 = scrp.tile([128, D], f32, tag="ttrscr")
                nc.vector.affine_mul_reduce(
                    out=s2[:], accum_out=d3all[:, t : t + 1],
                    in0=xt[:, k, :], in1=o3b[:], scale=1.0, bias=0.0)
                s3 = scrp.tile([128, D], f32, tag="actscr")
                nc.scalar.activation(
                    s3[:], xt[:, k, :], AF.Square, accum_out=ssall[:, t : t + 1])

        # ---- phase 2: exp-cosines ----
        nrm = statsp.tile([128, TPC], f32)
        nc.scalar.activation(nrm[:], ssall[:], AF.Sqrt)
        rs = statsp.tile([128, TPC], f32)
        nc.vector.reciprocal(rs[:], nrm[:])
        t2 = statsp.tile([128, TPC], f32)
        nc.vector.tensor_tensor(out=t2[:], in0=d2all[:], in1=rs[:], op=OP.mult)
        t3 = statsp.tile([128, TPC], f32)
        nc.vector.tensor_tensor(out=t3[:], in0=d3all[:], in1=rs[:], op=OP.mult)
        # eall[:, 0:64] = e2 per (row p, tile t); eall[:, 64:128] = e3
        eall = statsp.tile([128, 2 * TPC], f32)
        nc.scalar.activation(eall[:, 0:TPC], t2[:], AF.Exp, scale=invn2b[:])
        nc.scalar.activation(eall[:, TPC:], t3[:], AF.Exp, scale=invn3b[:])

        # ---- phase 3a: local totals -> post the AllGather as early as possible
        # per-(branch,tile) totals, row layout: totr[0, q] = sum_p eall[p, q]
        totr_ps = psump.tile([1, 128], f32)
        nc.tensor.matmul(totr_ps[:], ones128[:], eall[:], start=True, stop=True)
        totr = smallp.tile([1, 128], f32)
        nc.vector.tensor_copy(totr[:], totr_ps[:])
        tl = smallp.tile([1, 2], f32)
        nc.vector.tensor_reduce(out=tl[:, 0:1], in_=totr[:, 0:TPC], axis=AX.X, op=OP.add)
        nc.vector.tensor_reduce(out=tl[:, 1:2], in_=totr[:, TPC:], axis=AX.X, op=OP.add)
        cc_in = dramp.tile([1, 2], f32)
        cc_out = dramp.tile([8, 2], f32, addr_space="Shared")
        nc.sync.dma_start(cc_in[:], tl[:])
        nc.gpsimd.collective_compute(
            "AllGather", OP.bypass, replica_groups=[list(range(NCORES))],
            ins=[cc_in.opt()], outs=[cc_out.opt()])

        # ---- phase 3b: shard-local scans (overlap the AllGather skew wait)
        # transpose -> eT[q, p] with q = branch*64 + t
        eT_ps = psump.tile([128, 128], f32)
        nc.tensor.transpose(eT_ps[:], eall[:], ident[:])
        eT = statsp.tile([128, 128], f32)
        nc.scalar.copy(eT[:], eT_ps[:])
        # shifted (exclusive) tile totals, local only
        sh = smallp.tile([1, 128], f32)
        nc.vector.memset(sh[:, 0:1], 0.0)
        nc.vector.memset(sh[:, TPC : TPC + 1], 0.0)
        nc.vector.tensor_copy(sh[:, 1:TPC], totr[:, 0 : TPC - 1])
        nc.vector.tensor_copy(sh[:, TPC + 1 :], totr[:, TPC : 2 * TPC - 1])
        baser = smallp.tile([1, 128], f32)
        nc.vector.tensor_tensor_scan(
            out=baser[:, 0:TPC], data0=sh[:, 0:TPC], data1=sh[:, 0:TPC],
            initial=0.0, op0=OP.add, op1=OP.bypass)
        nc.vector.tensor_tensor_scan(
            out=baser[:, TPC:], data0=sh[:, TPC:], data1=sh[:, TPC:],
            initial=0.0, op0=OP.add, op1=OP.bypass)
        # move per-tile bases onto partitions: basec[q, 0] = baser[0, q]
        basec = smallp.tile([128, 1], f32)
        nc.sync.dma_start(basec[:], baser[:])
        # inclusive scan within each tile (along p) seeded by the local base:
        # sufl[q, p] = local suffix sums (missing only the global core base)
        sufl = statsp.tile([128, 128], f32)
        nc.vector.tensor_tensor_scan(
            out=sufl[:], data0=eT[:], data1=eT[:], initial=basec[:],
            op0=OP.add, op1=OP.bypass)

        # ---- phase 3c: consume the AllGather ----
        ag = smallp.tile([8, 2], f32)
        nc.sync.dma_start(ag[:], cc_out[:])
        # per-partition global bases: gb_ps[q, b] = sum_{c < my_core} tot_b[c]
        gb_ps = psump.tile([128, 2], f32)
        nc.tensor.matmul(gb_ps[:], mltt[:], ag[:], start=True, stop=True)
        tg_ps = psump.tile([1, 2], f32)
        nc.tensor.matmul(tg_ps[:], ones128[0:8, :], ag[:], start=True, stop=True)

        gb = smallp.tile([128, 2], f32)
        nc.vector.tensor_copy(gb[:], gb_ps[:])

        # ---- phase 4: log-reduction (global base folded into Ln bias) ----
        lnscr = statsp.tile([128, 128], f32)
        lnacc = smallp.tile([128, 1], f32)
        nc.scalar.activation(lnscr[0:TPC, :], sufl[0:TPC, :], AF.Ln,
                             bias=gb[0:TPC, 0:1], accum_out=lnacc[0:TPC, :])
        nc.scalar.activation(lnscr[TPC:, :], sufl[TPC:, :], AF.Ln,
                             bias=gb[TPC:, 1:2], accum_out=lnacc[TPC:, :])
        part_ps = psump.tile([1, 1], f32)
        nc.tensor.matmul(part_ps[:], ones128[:], lnacc[:], start=True, stop=True)

        # local partial = is0 * N * (log T2 + log T3) - sum(log suf)
        lt = smallp.tile([1, 2], f32)
        nc.scalar.activation(lt[:], tg_ps[:], AF.Ln)
        lts = smallp.tile([1, 1], f32)
        nc.vector.tensor_reduce(out=lts[:], in_=lt[:], axis=AX.X, op=OP.add)
        f1 = smallp.tile([1, 1], f32)
        nc.scalar.mul(f1[:], lts[:], float(N))
        f2 = smallp.tile([1, 1], f32)
        nc.vector.tensor_tensor(out=f2[:], in0=f1[:], in1=is0t[:], op=OP.mult)
        f3 = smallp.tile([1, 1], f32)
        nc.vector.tensor_tensor(out=f3[:], in0=f2[:], in1=part_ps[:], op=OP.subtract)

        # AllGather partials and reduce so every core emits the full loss
        cc2_in = dramp.tile([1, 1], f32)
        cc2_out = dramp.tile([8, 1], f32, addr_space="Shared")
        nc.sync.dma_start(cc2_in[:], f3[:])
        nc.gpsimd.collective_compute(
            "AllGather", OP.bypass, replica_groups=[list(range(NCORES))],
            ins=[cc2_in.opt()], outs=[cc2_out.opt()])
        agp = smallp.tile([8, 1], f32)
        nc.sync.dma_start(agp[:], cc2_out[:])
        fin_ps = psump.tile([1, 1], f32)
        nc.tensor.matmul(fin_ps[:], ones128[0:8, :], agp[:], start=True, stop=True)
        fin = smallp.tile([1, 1], f32)
        nc.vector.tensor_copy(fin[:], fin_ps[:])
        nc.sync.dma_start(loss_out[:], fin[:])


def build_nc():
    """Build + compile the SPMD Bass program (cached)."""
    global _compiled_nc
    if _compiled_nc is not None:
        return _compiled_nc
    import concourse.bacc as bacc
    import concourse.mybir as mybir
    from concourse import masks, tile

    f32 = mybir.dt.float32
    nc = bacc.Bacc("TRN2", target_bir_lowering=False, debug=False,
                   num_devices=NCORES)
    xs = nc.dram_tensor("xs", [SH, D], f32, kind="ExternalInput")
    o2 = nc.dram_tensor("o2", [1, D], f32, kind="ExternalInput")
    o3 = nc.dram_tensor("o3", [1, D], f32, kind="ExternalInput")
    mlt = nc.dram_tensor("mlt", [8, 128], f32, kind="ExternalInput")
    is0 = nc.dram_tensor("is0", [1, 1], f32, kind="ExternalInput")
    loss = nc.dram_tensor("loss", [1, 1], f32, kind="ExternalOutput")

    with tile.TileContext(nc) as tc:
        _body(tc, mybir, masks, xs.ap(), o2.ap(), o3.ap(), mlt.ap(), is0.ap(),
              loss.ap())
    nc.compile()
    _compiled_nc = nc
    return nc


def make_in_maps(output1, output2, output3, ranking):
    """Host-side shard: sort rows by descending ranking (stable, matching
    jnp.argsort(-ranking)), then feed in reversed (ascending) order so the
    device's forward cumsums are the reference's suffix sums."""
    ranking = np.asarray(ranking, dtype=np.float32)
    order = np.argsort(-ranking, kind="stable")
    rho = order[::-1]
    xs_full = np.ascontiguousarray(np.asarray(output1, dtype=np.float32)[rho])
    o2 = np.ascontiguousarray(np.asarray(output2, dtype=np.float32).reshape(1, D))
    o3 = np.ascontiguousarray(np.asarray(output3, dtype=np.float32).reshape(1, D))
    in_maps = []
    for c in range(NCORES):
        mlt = np.zeros((8, 128), np.float32)
        mlt[:c] = 1.0
        is0 = np.full((1, 1), 1.0 if c == 0 else 0.0, np.float32)
        in_maps.append({
            "xs": xs_full[c * SH : (c + 1) * SH],
            "o2": o2, "o3": o3, "mlt": mlt, "is0": is0,
        })
    return in_maps


def kernel(output1, output2, output3, ranking):
    from concourse.bass_utils import run_bass_kernel_spmd

    nc = build_nc()
    in_maps = make_in_maps(output1, output2, output3, ranking)
    res = run_bass_kernel_spmd(nc, in_maps, core_ids=list(range(NCORES)))
    out = res.results[0]["loss"]
    return np.asarray(out, dtype=np.float32).reshape(())
